# revision 27
# baseline (speedup 1.0000x reference)
"""Trainium2 Bass kernel for nn_MultiHeadAttention (B=4, T=2048, D=1024,
H=16, d_k=64) on 8 NeuronCores.

Sharding: tensor-parallel over heads - core c computes heads {2c, 2c+1} for
ALL batches (W_q/W_k/W_v column-sharded, W_o row-sharded). The final
all-reduce of the output projection is replaced by a host-side sum of the 8
partial outputs. Per-batch attention length (ceil(valid_len/128) Tk tiles)
is baked into the single SPMD program, keeping every core's instruction
stream identical AND load-balanced.

v2 rewrite (from trace analysis of the v1 baseline):
  - K/V loads + K/V projections trimmed to the valid length (the padding
    mask zeroes their contribution anyway): halves input DMA and V-proj PE.
  - scores^T layout (Tk on partitions, Tq on free): both heads' QK^T
    matmuls write adjacent PSUM banks of one [P, 2, 512] tile and a single
    1024-wide exp evacuates them (half the ACT instruction overhead).
    The padding mask is applied via the exp bias only on the tile that
    straddles valid_len; full tiles use bias 0.
  - softmax denominator from a ones-column folded into the P@V lhsT.
  - normalization: DVE reciprocal + GpSimd partition-broadcast + DVE mul
    (no PE broadcast matmuls, no staging DMAs; ACT does exp ONLY).
  - emission is globally interleaved: projection / output-projection
    chunks of other batches are woven between attention steps so the
    in-order PE stream never starves while ACT paces the softmax.
  - trn2 encodes at most one semaphore wait per instruction; a post-pass
    splits any multi-wait instruction Tile emits.
"""
import os
import sys

for _p in ("/opt/trn_rl_repo", "/root/.axon_site/_ro/trn_rl_repo"):
    if os.path.isdir(_p) and _p not in sys.path:
        sys.path.append(_p)

import numpy as np
import ml_dtypes

import concourse.bass as bass
import concourse.mybir as mybir
import concourse.tile as tile
from concourse.bass import ts
from concourse.bass_utils import run_bass_kernel_spmd

D = 1024
T = 2048
H = 16
DK = 64
P = 128
KC = D // P          # 8 contraction chunks for the projections
NT = T // 512        # 4 Tq/Tk 512-chunks ("windows")
TC = T // P          # 16 Tk tiles
NCORES = 8
MASK_NEG = -30000.0

F32 = mybir.dt.float32
F32R = mybir.dt.float32r
BF16 = mybir.dt.bfloat16
AF = mybir.ActivationFunctionType
BF16_NP = ml_dtypes.bfloat16
FP8E3 = mybir.dt.float8e3
FP8E3_NP = ml_dtypes.float8_e3m4


def _split_multi_waits(nc):
    """trn2 instructions encode at most one sync wait; split the rest into
    standalone single-wait event-semaphore ops."""
    n_split = 0
    for f in nc.m.functions:
        for blk in f.blocks:
            insts = blk.instructions
            out = []
            changed = False
            for inst in insts:
                si = inst.sync_info
                if si is not None and len(si.on_wait) > 1:
                    waits = list(si.on_wait)
                    for k, wt in enumerate(waits[:-1]):
                        ev = mybir.InstEventSemaphore(
                            name=f"{inst.name}_wsplit{k}",
                            engine=inst.engine,
                            ins=[],
                            outs=[],
                            bass_nofuse=True,
                            sync_info=mybir.SyncInfo(on_wait=[wt], on_update=[]),
                        )
                        out.append(ev)
                        n_split += 1
                    inst.sync_info = mybir.SyncInfo(
                        on_wait=[waits[-1]], on_update=si.on_update
                    )
                    changed = True
                out.append(inst)
            if changed:
                blk.instructions = out
    return n_split


def build_nc(NB, J_list, dt_x, dt_in):
    """Build the SPMD program.

    NB     : number of batch slots handled per core (slots sorted J asc)
    J_list : per batch slot, number of 128-row Tk tiles of attention
    dt_x   : dtype of weights/intermediates (BF16)
    dt_in  : dtype of the x inputs (fp8e3 halves the input DMA; the PE
             upconverts fp8/bf16 operands to fp22 internally)
    """
    CPB = P              # 2 heads -> 128 projection columns per core
    WK_list = [min(NT, -(-j * P // 512)) for j in J_list]
    nc = bass.Bass()

    # window-major layout: one [P, KC, 512] window is contiguous per
    # partition so each DMA needs only 128 descriptors
    xq_d = [nc.declare_dram_parameter(f"xq{s}", [NT, P, KC, 512], dt_in,
                                      isOutput=False) for s in range(NB)]
    xk_d = [nc.declare_dram_parameter(f"xk{s}", [WK_list[s], P, KC, 512],
                                      dt_in, isOutput=False) for s in range(NB)]
    xv_d = [nc.declare_dram_parameter(f"xv{s}", [WK_list[s], P, KC, 512],
                                      dt_in, isOutput=False) for s in range(NB)]
    wq_d = nc.declare_dram_parameter("wq", [P, KC, CPB], dt_x, isOutput=False)
    wk_d = nc.declare_dram_parameter("wk", [P, KC, CPB], dt_x, isOutput=False)
    wv_d = nc.declare_dram_parameter("wv", [P, KC, CPB], dt_x, isOutput=False)
    wo_d = nc.declare_dram_parameter("wo", [P, D], dt_x, isOutput=False)
    bq_d = nc.declare_dram_parameter("bq", [P, 1], F32, isOutput=False)
    bk_d = nc.declare_dram_parameter("bk", [P, 1], F32, isOutput=False)
    bv_d = nc.declare_dram_parameter("bv", [1, CPB], dt_x, isOutput=False)
    mk_d = nc.declare_dram_parameter("mk", [P, NB], F32, isOutput=False)
    sel_d = nc.declare_dram_parameter("sel", [2, P], F32R, isOutput=False)
    o_d = [nc.declare_dram_parameter(f"o{s}", [T, D], BF16, isOutput=True)
           for s in range(NB)]

    with tile.TileContext(nc) as tc:
        with (
            tc.tile_pool(name="pers", bufs=1) as pers,
            tc.tile_pool(name="sxq", bufs=4) as sxq,
            tc.tile_pool(name="sxk", bufs=3) as sxk,
            tc.tile_pool(name="sxv", bufs=3) as sxv,
            tc.tile_pool(name="attn", bufs=4) as attn_pool,
            tc.tile_pool(name="rcp", bufs=4) as rcp,
            tc.tile_pool(name="dns", bufs=6) as dns,
            tc.tile_pool(name="rcs", bufs=6) as rcs,
            tc.tile_pool(name="outp", bufs=4) as outp,
            tc.tile_pool(name="ps_sc", bufs=2, space="PSUM") as ps_sc,
            tc.tile_pool(name="ps_pv", bufs=2, space="PSUM") as ps_pv,
            tc.tile_pool(name="ps_pj", bufs=2, space="PSUM") as ps_pj,
        ):
            # ---- persistent tensors -------------------------------------
            wq = pers.tile([P, KC, CPB], dt_x, name="wq")
            wk = pers.tile([P, KC, CPB], dt_x, name="wk")
            wv = pers.tile([P, KC, CPB], dt_x, name="wv")
            wo = pers.tile([P, D], dt_x, name="wo")
            bq = pers.tile([P, 1], F32, name="bq")
            bk = pers.tile([P, 1], F32, name="bk")
            bv = pers.tile([1, CPB], dt_x, name="bv")
            mk = pers.tile([P, NB], F32, name="mk")
            nc.sync.dma_start(wq[:], wq_d[:])
            nc.sync.dma_start(wk[:], wk_d[:])
            nc.sync.dma_start(wv[:], wv_d[:])
            nc.sync.dma_start(wo[:], wo_d[:])
            nc.sync.dma_start(bq[:], bq_d[:])
            nc.sync.dma_start(bk[:], bk_d[:])
            nc.sync.dma_start(bv[:], bv_d[:])
            nc.sync.dma_start(mk[:], mk_d[:])

            ones_t = pers.tile([1, P], dt_x, name="ones_t")  # V-bias fold lhsT
            nc.vector.memset(ones_t[:], 1.0)
            sel = pers.tile([2, P], F32R, name="sel")  # 1/den bcast lhsT
            nc.sync.dma_start(sel[:], sel_d[:])

            QT = [pers.tile([P, T], dt_x, name=f"QT{s}") for s in range(NB)]
            KT = [pers.tile([P, WK_list[s] * 512], dt_x, name=f"KT{s}")
                  for s in range(NB)]
            # V with a ones column folded in at free index DK of each head
            V = [pers.tile([P, J_list[s], 2, DK + 1], dt_x, name=f"V{s}")
                 for s in range(NB)]
            for s in range(NB):
                nc.vector.memset(V[s][:, :, :, DK], 1.0)

            uo = [pers.tile([P, NT, 512], BF16, name=f"uo{s}")
                  for s in range(NB)]
            AO = [pers.tile([P, T], dt_x, name=f"AO{s}") for s in range(NB)]

            xq_t, xk_t, xv_t, denst = {}, {}, {}, {}

            # ---- chunk emitters -----------------------------------------
            def load(s, w):
                def go():
                    if w < NT:
                        t = sxq.tile([P, KC, 512], dt_in, tag="xq", name="xqw")
                        nc.sync.dma_start(t[:], xq_d[s][w])
                        xq_t[(s, w)] = t
                    if w < WK_list[s]:
                        t = sxk.tile([P, KC, 512], dt_in, tag="xk", name="xkw")
                        nc.sync.dma_start(t[:], xk_d[s][w])
                        xk_t[(s, w)] = t
                        t = sxv.tile([P, KC, 512], dt_in, tag="xv", name="xvw")
                        nc.sync.dma_start(t[:], xv_d[s][w])
                        xv_t[(s, w)] = t
                return go

            def proj_q(s, w):
                def go():
                    xw = xq_t.pop((s, w))
                    ps = ps_pj.tile([P, 512], F32, tag="pj", name="psq")
                    for kc in range(KC):
                        nc.tensor.matmul(ps[:], wq[:, kc, :], xw[:, kc, :],
                                         start=(kc == 0), stop=(kc == KC - 1))
                    nc.vector.tensor_scalar_add(QT[s][:, ts(w, 512)], ps[:],
                                                bq[:, 0:1])
                return go

            def proj_k(s, w):
                def go():
                    xw = xk_t.pop((s, w))
                    ps = ps_pj.tile([P, 512], F32, tag="pj", name="psk")
                    for kc in range(KC):
                        nc.tensor.matmul(ps[:], wk[:, kc, :], xw[:, kc, :],
                                         start=(kc == 0), stop=(kc == KC - 1))
                    nc.vector.tensor_scalar_add(KT[s][:, ts(w, 512)], ps[:],
                                                bk[:, 0:1])
                return go

            def proj_v(s, m):
                def go():
                    xw = xv_t[(s, m // 4)]
                    last_in_w = (m % 4 == 3) or (m == J_list[s] - 1)
                    ps = ps_pj.tile([P, 512], F32, tag="pj", name="psv")
                    pv_ = ps[:, 0:CPB]
                    for kc in range(KC):
                        nc.tensor.matmul(pv_, xw[:, kc, ts(m % 4, P)],
                                         wv[:, kc, :],
                                         start=(kc == 0), stop=False)
                    nc.tensor.matmul(pv_, ones_t[0:1, :], bv[0:1, :],
                                     start=False, stop=True)
                    nc.vector.tensor_copy(
                        V[s][:, m, :, 0:DK],
                        ps[:, 0:CPB].rearrange("p (h d) -> p h d", d=DK))
                    if last_in_w:
                        xv_t.pop((s, m // 4))
                return go

            def attn_gen(s, tq):
                # generator: yields after each pipeline step
                J = J_list[s]
                at = {}

                def qk(j):
                    sc = ps_sc.tile([P, 2, 512], F32, tag="sc", name="sc")
                    nc.tensor.matmul(sc[:, 0, :], KT[s][0:DK, ts(j, P)],
                                     QT[s][0:DK, ts(tq, 512)],
                                     start=True, stop=True,
                                     tile_position=(0, 0))
                    nc.tensor.matmul(sc[:, 1, :], KT[s][DK:P, ts(j, P)],
                                     QT[s][DK:P, ts(tq, 512)],
                                     start=True, stop=True,
                                     tile_position=(DK, 0))
                    a = attn_pool.tile([P, 2, 512], dt_x, tag="at", name="at")
                    bias = mk[:, s:s + 1] if j == J - 1 else 0.0
                    nc.scalar.activation(a[:], sc[:], AF.Exp,
                                         bias=bias, scale=0.125)
                    at[j] = a

                pv0 = ps_pv.tile([P, 512], F32, tag="pv", name="pv0")
                pv1 = ps_pv.tile([P, 512], F32, tag="pv", name="pv1")
                qk(0)
                if J > 1:
                    qk(1)
                yield
                for j in range(J):
                    a = at.pop(j)
                    nc.tensor.matmul(pv0[0:DK + 1, :], V[s][:, j, 0, :],
                                     a[:, 0, :],
                                     start=(j == 0), stop=(j == J - 1))
                    nc.tensor.matmul(pv1[0:DK + 1, :], V[s][:, j, 1, :],
                                     a[:, 1, :],
                                     start=(j == 0), stop=(j == J - 1))
                    if j + 2 < J:
                        qk(j + 2)
                    yield
                # evacuate denominators first (their staging DMAs need
                # time to land before the deferred norm), then the outputs
                dt0 = rcp.tile([1, 512], F32, tag="rt", name="dt0")
                dt1 = rcp.tile([1, 512], F32, tag="rt", name="dt1")
                nc.vector.tensor_copy(dt0[0:1, :], pv0[DK:DK + 1, :])
                nc.vector.tensor_copy(dt1[0:1, :], pv1[DK:DK + 1, :])
                dn = dns.tile([2, 512], F32, tag="dn", name="dn")
                nc.sync.dma_start(dn[0:1, :], dt0[0:1, :])
                nc.sync.dma_start(dn[1:2, :], dt1[0:1, :])
                denst[(s, tq)] = dn
                nc.vector.tensor_copy(uo[s][0:DK, tq, :], pv0[0:DK, :])
                nc.vector.tensor_copy(uo[s][DK:P, tq, :], pv1[0:DK, :])
                yield

            def norm(s, tq):
                def go():
                    dn = denst.pop((s, tq))
                    rc = rcs.tile([2, 512], F32R, tag="rc", name="rc")
                    with nc.allow_low_precision(
                            reason="f32r output is bit-identical to f32"):
                        nc.vector.reciprocal(rc[0:2, :], dn[0:2, :])
                    psb = ps_pj.tile([P, 512], F32, tag="pj", name="psb")
                    nc.tensor.matmul(psb[:], sel[0:2, :], rc[0:2, :],
                                     start=True, stop=True)
                    nc.vector.tensor_mul(AO[s][:, ts(tq, 512)],
                                         uo[s][:, tq, :], psb[:])
                return go

            def outproj(s, m):
                def go():
                    ot = outp.tile([P, D], BF16, tag="ot", name="ot")
                    for n2 in range(2):
                        ps = ps_pj.tile([P, 512], F32, tag="pj", name="pso")
                        nc.tensor.matmul(ps[:], AO[s][:, ts(m, P)],
                                         wo[:, ts(n2, 512)],
                                         start=True, stop=True)
                        if (2 * m + n2) % 4 == 0:
                            nc.scalar.activation(ot[:, ts(n2, 512)], ps[:],
                                                 AF.Identity)
                        else:
                            nc.vector.tensor_copy(ot[:, ts(n2, 512)], ps[:])
                    nc.sync.dma_start(o_d[s][ts(m, P), :], ot[:])
                return go

            # ---- interleaved emission -----------------------------------
            # Fillers (projection / normalization / output-projection work
            # of other batches) are woven between attention pipeline steps.
            # Each attention step force-drains exactly the proj chunks it
            # depends on (requirement gating), the rest is paced evenly.
            fillers = []
            mk_pq, mk_pk, mk_pv = {}, {}, {}

            def append_filler(ch, d=None, key=None):
                fillers.append(ch)
                if d is not None:
                    d[key] = len(fillers)

            def append_proj(s, skip_w0=False):
                for w in range(NT):
                    if w == 0 and skip_w0:
                        continue
                    append_filler(proj_q(s, w), mk_pq, (s, w))
                    if w < WK_list[s]:
                        append_filler(proj_k(s, w), mk_pk, (s, w))
                        for m in range(4 * w, min(4 * w + 4, J_list[s])):
                            append_filler(proj_v(s, m), mk_pv, (s, m))

            def load_chunks(s):
                return [load(s, w) for w in range(NT)]

            # bootstrap: loads of slots 0/1 + first window-group of slot 0
            for c in load_chunks(0):
                c()
            if NB > 1:
                for c in load_chunks(1):
                    c()
            proj_q(0, 0)()
            proj_k(0, 0)()
            for m in range(0, min(4, J_list[0])):
                proj_v(0, m)()
            append_proj(0, skip_w0=True)
            if NB > 1:
                append_proj(1)

            total_attn = sum(4 * (J_list[s] + 2) for s in range(NB))
            done_attn = 0
            acc = 0.0
            fi = 0

            def drain_to(idx):
                nonlocal fi
                while fi < min(idx, len(fillers)):
                    fillers[fi]()
                    fi += 1

            pending = []  # (s, tq) whose norm+outproj await the next tq
            for s in range(NB):
                J = J_list[s]
                if s + 2 < NB:
                    for c in load_chunks(s + 2):
                        fillers.append(c)
                    append_proj(s + 2)
                for tq in range(NT):
                    keep = 0 if (s == NB - 1 and tq == NT - 1) else 1
                    while len(pending) > keep:
                        ps_, ptq = pending.pop(0)
                        fillers.append(norm(ps_, ptq))
                        for m in range(4 * ptq, 4 * ptq + 4):
                            fillers.append(outproj(ps_, m))
                    gen = attn_gen(s, tq)
                    for step in range(J + 2):
                        if step == 0:
                            req = [mk_pq.get((s, tq), 0),
                                   mk_pk.get((s, 0), 0)]
                        elif step <= J:
                            j = step - 1  # emits pv(j) then qk(j+2)
                            req = [mk_pv.get((s, j), 0)]
                            if j + 2 < J:
                                req.append(mk_pk.get((s, (j + 2) // 4), 0))
                        else:
                            req = []
                        drain_to(max(req, default=0))
                        next(gen)
                        done_attn += 1
                        acc += 1.3 * (len(fillers) - fi) / max(
                            1, total_attn - done_attn)
                        n = int(acc)
                        acc -= n
                        for _ in range(n):
                            if fi < len(fillers):
                                fillers[fi]()
                                fi += 1
                    pending.append((s, tq))
            while pending:
                ps_, ptq = pending.pop(0)
                fillers.append(norm(ps_, ptq))
                for m in range(4 * ptq, 4 * ptq + 4):
                    fillers.append(outproj(ps_, m))
            drain_to(len(fillers))

    _split_multi_waits(nc)
    return nc


_CACHE = {}


def _get_nc(NB, J_list, dt_x, dt_in):
    key = (NB, tuple(J_list), str(dt_x), str(dt_in))
    if key not in _CACHE:
        _CACHE[key] = build_nc(NB, J_list, dt_x, dt_in)
    return _CACHE[key]


def _xt(x, dt_np, nw):
    """[T, D] -> [nw, P, KC, 512] transposed window-major layout."""
    xt = x.T.reshape(KC, P, NT, 512).transpose(2, 1, 0, 3)[:nw]
    return np.ascontiguousarray(xt).astype(dt_np)


def kernel(**inputs):
    query = np.asarray(inputs["query"], dtype=np.float32)
    key = np.asarray(inputs["key"], dtype=np.float32)
    value = np.asarray(inputs["value"], dtype=np.float32)
    vl = np.asarray(inputs["valid_length"]).astype(np.int64)
    W_q = np.asarray(inputs["W_q"], dtype=np.float32)
    b_q = np.asarray(inputs["b_q"], dtype=np.float32)
    W_k = np.asarray(inputs["W_k"], dtype=np.float32)
    b_k = np.asarray(inputs["b_k"], dtype=np.float32)
    W_v = np.asarray(inputs["W_v"], dtype=np.float32)
    b_v = np.asarray(inputs["b_v"], dtype=np.float32)
    W_o = np.asarray(inputs["W_o"], dtype=np.float32)
    b_o = np.asarray(inputs["b_o"], dtype=np.float32)

    B = query.shape[0]
    NB = B
    CPB = (H // NCORES) * DK       # 2 heads per core -> 128 cols
    dt_x = BF16
    dt_np = BF16_NP
    dt_in = FP8E3
    din_np = FP8E3_NP

    # slot s handles batch order[s]; J (Tk tiles) baked per slot, asc order
    order = np.argsort(vl, kind="stable")
    # vl==0 -> uniform attention over all T keys (q zeroed); sort those last
    order = np.concatenate([order[vl[order] != 0], order[vl[order] == 0]])
    J_list = []
    for s in range(NB):
        v = int(vl[order[s]])
        J_list.append(TC if v == 0 else max(1, -(-v // P)))
    WK_list = [min(NT, -(-j * P // 512)) for j in J_list]

    nc = _get_nc(NB, J_list, dt_x, dt_in)

    # host-side shard prep (shared across cores)
    xq_np, xk_np, xv_np = [], [], []
    mk_np = np.zeros((P, NB), np.float32)
    for s in range(NB):
        b = int(order[s])
        v = int(vl[b])
        q_b = query[b] if v != 0 else np.zeros_like(query[b])
        xq_np.append(_xt(q_b, din_np, NT))
        xk_np.append(_xt(key[b], din_np, WK_list[s]))
        xv_np.append(_xt(value[b], din_np, WK_list[s]))
        if v != 0:
            # mask bias for the last Tk tile (rows j*128+p >= v)
            rows = (J_list[s] - 1) * P + np.arange(P)
            mk_np[:, s] = np.where(rows < v, 0.0, MASK_NEG)

    sel_np = np.zeros((2, P), np.float32)
    sel_np[0, 0:DK] = 1.0
    sel_np[1, DK:P] = 1.0
    in_maps = []
    for c in range(NCORES):
        c0 = c * CPB
        cols = slice(c0, c0 + CPB)
        im = {
            "wq": np.ascontiguousarray(
                W_q.reshape(KC, P, H * DK).transpose(1, 0, 2)[:, :, cols]
            ).astype(dt_np),
            "wk": np.ascontiguousarray(
                W_k.reshape(KC, P, H * DK).transpose(1, 0, 2)[:, :, cols]
            ).astype(dt_np),
            "wv": np.ascontiguousarray(
                W_v.reshape(KC, P, H * DK).transpose(1, 0, 2)[:, :, cols]
            ).astype(dt_np),
            "wo": np.ascontiguousarray(W_o[cols]).astype(dt_np),
            "bq": np.ascontiguousarray(b_q[cols][:, None]).astype(np.float32),
            "bk": np.ascontiguousarray(b_k[cols][:, None]).astype(np.float32),
            "bv": np.ascontiguousarray(b_v[cols][None, :]).astype(dt_np),
            "mk": mk_np,
            "sel": sel_np,
        }
        for s in range(NB):
            im[f"xq{s}"] = xq_np[s]
            im[f"xk{s}"] = xk_np[s]
            im[f"xv{s}"] = xv_np[s]
        in_maps.append(im)

    res = run_bass_kernel_spmd(nc, in_maps, list(range(NCORES)))

    out = np.zeros((B, T, D), np.float32)
    for s in range(NB):
        b = int(order[s])
        acc = np.zeros((T, D), np.float32)
        for c in range(NCORES):
            acc += np.asarray(res.results[c][f"o{s}"]).astype(np.float32)
        out[b] = acc + b_o[None, :]
    return out


# revision 28
# speedup vs baseline: 1.0262x; 1.0262x over previous
"""Trainium2 Bass kernel for nn_MultiHeadAttention (B=4, T=2048, D=1024,
H=16, d_k=64) on 8 NeuronCores.

Sharding: tensor-parallel over heads - core c computes heads {2c, 2c+1} for
ALL batches (W_q/W_k/W_v column-sharded, W_o row-sharded). The final
all-reduce of the output projection is replaced by a host-side sum of the 8
partial outputs. Per-batch attention length (ceil(valid_len/128) Tk tiles)
is baked into the single SPMD program, keeping every core's instruction
stream identical AND load-balanced.

Design (from trace analysis of the v1 baseline):
  - K/V loads + K/V projections trimmed to the valid length (the padding
    mask zeroes their contribution anyway): halves input DMA and V-proj PE.
  - scores^T layout (Tk on partitions, Tq on free): both heads' QK^T
    matmuls write adjacent PSUM banks of one [P, 2, 512] tile and a single
    1024-wide exp evacuates them (half the ACT instruction overhead).
    The padding mask is applied via the exp bias only on the tile that
    straddles valid_len; full tiles use bias 0.
  - softmax denominator from a ones-column folded into the P@V lhsT.
  - x inputs in fp8e3 (e3m4): halves the input HBM traffic; the PE
    upconverts mixed fp8/bf16 operands internally (weights stay bf16).
  - normalization is deferred two Tq-chunks: den rows stage to SBUF via
    tiny DMAs, then a batched DVE reciprocal, one K=2 select-matrix
    matmul broadcasts 1/den across partitions, one DVE mul normalizes.
  - emission is globally interleaved: projection / output-projection
    chunks of other batches are woven between attention steps so the
    in-order PE stream never starves while ACT paces the softmax.
  - trn2 encodes at most one semaphore wait per instruction; a post-pass
    splits any multi-wait instruction Tile emits.
"""
import os
import sys

for _p in ("/opt/trn_rl_repo", "/root/.axon_site/_ro/trn_rl_repo"):
    if os.path.isdir(_p) and _p not in sys.path:
        sys.path.append(_p)

import numpy as np
import ml_dtypes

import concourse.bass as bass
import concourse.mybir as mybir
import concourse.tile as tile
from concourse.bass import ts
from concourse.bass_utils import run_bass_kernel_spmd

D = 1024
T = 2048
H = 16
DK = 64
P = 128
KC = D // P          # 8 contraction chunks for the projections
NT = T // 512        # 4 Tq/Tk 512-chunks ("windows")
TC = T // P          # 16 Tk tiles
NCORES = 8
MASK_NEG = -30000.0

F32 = mybir.dt.float32
F32R = mybir.dt.float32r
BF16 = mybir.dt.bfloat16
AF = mybir.ActivationFunctionType
BF16_NP = ml_dtypes.bfloat16
FP8E3 = mybir.dt.float8e3
FP8E3_NP = ml_dtypes.float8_e3m4


def _split_multi_waits(nc):
    """trn2 instructions encode at most one sync wait; split the rest into
    standalone single-wait event-semaphore ops."""
    n_split = 0
    for f in nc.m.functions:
        for blk in f.blocks:
            insts = blk.instructions
            out = []
            changed = False
            for inst in insts:
                si = inst.sync_info
                if si is not None and len(si.on_wait) > 1:
                    waits = list(si.on_wait)
                    for k, wt in enumerate(waits[:-1]):
                        ev = mybir.InstEventSemaphore(
                            name=f"{inst.name}_wsplit{k}",
                            engine=inst.engine,
                            ins=[],
                            outs=[],
                            bass_nofuse=True,
                            sync_info=mybir.SyncInfo(on_wait=[wt], on_update=[]),
                        )
                        out.append(ev)
                        n_split += 1
                    inst.sync_info = mybir.SyncInfo(
                        on_wait=[waits[-1]], on_update=si.on_update
                    )
                    changed = True
                out.append(inst)
            if changed:
                blk.instructions = out
    return n_split


def build_nc(NB, J_list, dt_x, dt_in):
    """Build the SPMD program.

    NB     : number of batch slots handled per core (slots sorted J asc)
    J_list : per batch slot, number of 128-row Tk tiles of attention
    dt_x   : dtype of weights/intermediates (BF16)
    dt_in  : dtype of the x inputs (fp8e3 halves the input DMA; the PE
             upconverts fp8/bf16 operands to fp22 internally)
    """
    CPB = P              # 2 heads -> 128 projection columns per core
    WK_list = [min(NT, -(-j * P // 512)) for j in J_list]
    nc = bass.Bass()

    # window-major layout: one [P, KC, 512] window is contiguous per
    # partition so each DMA needs only 128 descriptors
    xq_d = [nc.declare_dram_parameter(f"xq{s}", [NT, P, KC, 512], dt_in,
                                      isOutput=False) for s in range(NB)]
    xk_d = [nc.declare_dram_parameter(f"xk{s}", [WK_list[s], P, KC, 512],
                                      dt_in, isOutput=False) for s in range(NB)]
    xv_d = [nc.declare_dram_parameter(f"xv{s}", [WK_list[s], P, KC, 512],
                                      dt_in, isOutput=False) for s in range(NB)]
    wq_d = nc.declare_dram_parameter("wq", [P, KC, CPB], dt_x, isOutput=False)
    wk_d = nc.declare_dram_parameter("wk", [P, KC, CPB], dt_x, isOutput=False)
    wv_d = nc.declare_dram_parameter("wv", [P, KC, CPB], dt_x, isOutput=False)
    wo_d = nc.declare_dram_parameter("wo", [P, D], dt_x, isOutput=False)
    bq_d = nc.declare_dram_parameter("bq", [P, 1], F32, isOutput=False)
    bk_d = nc.declare_dram_parameter("bk", [P, 1], F32, isOutput=False)
    bv_d = nc.declare_dram_parameter("bv", [1, CPB], dt_x, isOutput=False)
    mk_d = nc.declare_dram_parameter("mk", [P, NB], F32, isOutput=False)
    sel_d = nc.declare_dram_parameter("sel", [2, P], F32R, isOutput=False)
    o_d = [nc.declare_dram_parameter(f"o{s}", [T, D], BF16, isOutput=True)
           for s in range(NB)]

    with tile.TileContext(nc) as tc:
        with (
            tc.tile_pool(name="pers", bufs=1) as pers,
            tc.tile_pool(name="sxq", bufs=4) as sxq,
            tc.tile_pool(name="sxk", bufs=3) as sxk,
            tc.tile_pool(name="sxv", bufs=3) as sxv,
            tc.tile_pool(name="attn", bufs=4) as attn_pool,
            tc.tile_pool(name="rcp", bufs=4) as rcp,
            tc.tile_pool(name="dns", bufs=6) as dns,
            tc.tile_pool(name="rcs", bufs=6) as rcs,
            tc.tile_pool(name="outp", bufs=4) as outp,
            tc.tile_pool(name="ps_sc", bufs=2, space="PSUM") as ps_sc,
            tc.tile_pool(name="ps_pv", bufs=2, space="PSUM") as ps_pv,
            tc.tile_pool(name="ps_pj", bufs=2, space="PSUM") as ps_pj,
        ):
            # ---- persistent tensors -------------------------------------
            wq = pers.tile([P, KC, CPB], dt_x, name="wq")
            wk = pers.tile([P, KC, CPB], dt_x, name="wk")
            wv = pers.tile([P, KC, CPB], dt_x, name="wv")
            wo = pers.tile([P, D], dt_x, name="wo")
            bq = pers.tile([P, 1], F32, name="bq")
            bk = pers.tile([P, 1], F32, name="bk")
            bv = pers.tile([1, CPB], dt_x, name="bv")
            mk = pers.tile([P, NB], F32, name="mk")
            nc.sync.dma_start(wq[:], wq_d[:])
            nc.sync.dma_start(wk[:], wk_d[:])
            nc.sync.dma_start(wv[:], wv_d[:])
            nc.sync.dma_start(wo[:], wo_d[:])
            nc.sync.dma_start(bq[:], bq_d[:])
            nc.sync.dma_start(bk[:], bk_d[:])
            nc.sync.dma_start(bv[:], bv_d[:])
            nc.sync.dma_start(mk[:], mk_d[:])

            ones_t = pers.tile([1, P], dt_x, name="ones_t")  # V-bias fold lhsT
            nc.vector.memset(ones_t[:], 1.0)
            sel = pers.tile([2, P], F32R, name="sel")  # 1/den bcast lhsT
            nc.sync.dma_start(sel[:], sel_d[:])

            QT = [pers.tile([P, T], dt_x, name=f"QT{s}") for s in range(NB)]
            KT = [pers.tile([P, WK_list[s] * 512], dt_x, name=f"KT{s}")
                  for s in range(NB)]
            # V with a ones column folded in at free index DK of each head
            V = [pers.tile([P, J_list[s], 2, DK + 1], dt_x, name=f"V{s}")
                 for s in range(NB)]
            for s in range(NB):
                nc.vector.memset(V[s][:, :, :, DK], 1.0)

            uo = [pers.tile([P, NT, 512], BF16, name=f"uo{s}")
                  for s in range(NB)]
            AO = [pers.tile([P, T], dt_x, name=f"AO{s}") for s in range(NB)]

            xq_t, xk_t, xv_t, denst = {}, {}, {}, {}

            # ---- chunk emitters -----------------------------------------
            def load(s, w):
                def go():
                    if w < NT:
                        t = sxq.tile([P, KC, 512], dt_in, tag="xq", name="xqw")
                        nc.sync.dma_start(t[:], xq_d[s][w])
                        xq_t[(s, w)] = t
                    if w < WK_list[s]:
                        t = sxk.tile([P, KC, 512], dt_in, tag="xk", name="xkw")
                        nc.sync.dma_start(t[:], xk_d[s][w])
                        xk_t[(s, w)] = t
                        t = sxv.tile([P, KC, 512], dt_in, tag="xv", name="xvw")
                        nc.sync.dma_start(t[:], xv_d[s][w])
                        xv_t[(s, w)] = t
                return go

            def proj_q(s, w):
                def go():
                    xw = xq_t.pop((s, w))
                    ps = ps_pj.tile([P, 512], F32, tag="pj", name="psq")
                    for kc in range(KC):
                        nc.tensor.matmul(ps[:], wq[:, kc, :], xw[:, kc, :],
                                         start=(kc == 0), stop=(kc == KC - 1))
                    nc.vector.tensor_scalar_add(QT[s][:, ts(w, 512)], ps[:],
                                                bq[:, 0:1])
                return go

            def proj_k(s, w):
                def go():
                    xw = xk_t.pop((s, w))
                    ps = ps_pj.tile([P, 512], F32, tag="pj", name="psk")
                    for kc in range(KC):
                        nc.tensor.matmul(ps[:], wk[:, kc, :], xw[:, kc, :],
                                         start=(kc == 0), stop=(kc == KC - 1))
                    nc.vector.tensor_scalar_add(KT[s][:, ts(w, 512)], ps[:],
                                                bk[:, 0:1])
                return go

            def proj_v(s, m):
                def go():
                    xw = xv_t[(s, m // 4)]
                    last_in_w = (m % 4 == 3) or (m == J_list[s] - 1)
                    ps = ps_pj.tile([P, 512], F32, tag="pj", name="psv")
                    pv_ = ps[:, 0:CPB]
                    for kc in range(KC):
                        nc.tensor.matmul(pv_, xw[:, kc, ts(m % 4, P)],
                                         wv[:, kc, :],
                                         start=(kc == 0), stop=False)
                    nc.tensor.matmul(pv_, ones_t[0:1, :], bv[0:1, :],
                                     start=False, stop=True)
                    nc.vector.tensor_copy(
                        V[s][:, m, :, 0:DK],
                        ps[:, 0:CPB].rearrange("p (h d) -> p h d", d=DK))
                    if last_in_w:
                        xv_t.pop((s, m // 4))
                return go

            def attn_gen(s, tq):
                # generator: yields after each pipeline step
                J = J_list[s]
                at = {}

                def qk(j):
                    sc = ps_sc.tile([P, 2, 512], F32, tag="sc", name="sc")
                    nc.tensor.matmul(sc[:, 0, :], KT[s][0:DK, ts(j, P)],
                                     QT[s][0:DK, ts(tq, 512)],
                                     start=True, stop=True,
                                     tile_position=(0, 0))
                    nc.tensor.matmul(sc[:, 1, :], KT[s][DK:P, ts(j, P)],
                                     QT[s][DK:P, ts(tq, 512)],
                                     start=True, stop=True,
                                     tile_position=(DK, 0))
                    a = attn_pool.tile([P, 2, 512], dt_x, tag="at", name="at")
                    bias = mk[:, s:s + 1] if j == J - 1 else 0.0
                    nc.scalar.activation(a[:], sc[:], AF.Exp,
                                         bias=bias, scale=0.125)
                    at[j] = a

                pv0 = ps_pv.tile([P, 512], F32, tag="pv", name="pv0")
                pv1 = ps_pv.tile([P, 512], F32, tag="pv", name="pv1")
                qk(0)
                if J > 1:
                    qk(1)
                yield
                for j in range(J):
                    a = at.pop(j)
                    nc.tensor.matmul(pv0[0:DK + 1, :], V[s][:, j, 0, :],
                                     a[:, 0, :],
                                     start=(j == 0), stop=(j == J - 1))
                    nc.tensor.matmul(pv1[0:DK + 1, :], V[s][:, j, 1, :],
                                     a[:, 1, :],
                                     start=(j == 0), stop=(j == J - 1))
                    if j + 2 < J:
                        qk(j + 2)
                    yield
                # evacuate denominators first (their staging DMAs need
                # time to land before the deferred norm), then the outputs
                dt0 = rcp.tile([1, 512], F32, tag="rt", name="dt0")
                dt1 = rcp.tile([1, 512], F32, tag="rt", name="dt1")
                nc.vector.tensor_copy(dt0[0:1, :], pv0[DK:DK + 1, :])
                nc.vector.tensor_copy(dt1[0:1, :], pv1[DK:DK + 1, :])
                dn = dns.tile([2, 512], F32, tag="dn", name="dn")
                nc.sync.dma_start(dn[0:1, :], dt0[0:1, :])
                nc.sync.dma_start(dn[1:2, :], dt1[0:1, :])
                denst[(s, tq)] = dn
                nc.vector.tensor_copy(uo[s][0:DK, tq, :], pv0[0:DK, :])
                nc.vector.tensor_copy(uo[s][DK:P, tq, :], pv1[0:DK, :])
                yield

            def norm(s, tq):
                def go():
                    dn = denst.pop((s, tq))
                    rc = rcs.tile([2, 512], F32R, tag="rc", name="rc")
                    with nc.allow_low_precision(
                            reason="f32r output is bit-identical to f32"):
                        nc.vector.reciprocal(rc[0:2, :], dn[0:2, :])
                    psb = ps_pj.tile([P, 512], F32, tag="pj", name="psb")
                    nc.tensor.matmul(psb[:], sel[0:2, :], rc[0:2, :],
                                     start=True, stop=True)
                    nc.vector.tensor_mul(AO[s][:, ts(tq, 512)],
                                         uo[s][:, tq, :], psb[:])
                return go

            def outproj(s, m):
                def go():
                    ot = outp.tile([P, D], BF16, tag="ot", name="ot")
                    for n2 in range(2):
                        ps = ps_pj.tile([P, 512], F32, tag="pj", name="pso")
                        nc.tensor.matmul(ps[:], AO[s][:, ts(m, P)],
                                         wo[:, ts(n2, 512)],
                                         start=True, stop=True)
                        if (2 * m + n2) % 4 == 0:
                            nc.scalar.activation(ot[:, ts(n2, 512)], ps[:],
                                                 AF.Identity)
                        else:
                            nc.vector.tensor_copy(ot[:, ts(n2, 512)], ps[:])
                    nc.sync.dma_start(o_d[s][ts(m, P), :], ot[:])
                return go

            # ---- interleaved emission -----------------------------------
            # Fillers (projection / normalization / output-projection work
            # of other batches) are woven between attention pipeline steps.
            # Each attention step force-drains exactly the proj chunks it
            # depends on (requirement gating), the rest is paced evenly.
            fillers = []
            mk_pq, mk_pk, mk_pv = {}, {}, {}

            def append_filler(ch, d=None, key=None):
                fillers.append(ch)
                if d is not None:
                    d[key] = len(fillers)

            def append_proj(s, skip_w0=False):
                for w in range(NT):
                    if w == 0 and skip_w0:
                        continue
                    append_filler(proj_q(s, w), mk_pq, (s, w))
                    if w < WK_list[s]:
                        append_filler(proj_k(s, w), mk_pk, (s, w))
                        for m in range(4 * w, min(4 * w + 4, J_list[s])):
                            append_filler(proj_v(s, m), mk_pv, (s, m))

            def load_chunks(s):
                return [load(s, w) for w in range(NT)]

            # bootstrap: loads of slots 0/1 + first window-group of slot 0
            for c in load_chunks(0):
                c()
            if NB > 1:
                for c in load_chunks(1):
                    c()
            proj_q(0, 0)()
            proj_k(0, 0)()
            for m in range(0, min(4, J_list[0])):
                proj_v(0, m)()
            append_proj(0, skip_w0=True)
            if NB > 1:
                append_proj(1)

            total_attn = sum(4 * (J_list[s] + 2) for s in range(NB))
            done_attn = 0
            acc = 0.0
            fi = 0

            def drain_to(idx):
                nonlocal fi
                while fi < min(idx, len(fillers)):
                    fillers[fi]()
                    fi += 1

            pending = []  # (s, tq) whose norm+outproj await the next tq
            for s in range(NB):
                J = J_list[s]
                if s + 2 < NB:
                    for c in load_chunks(s + 2):
                        fillers.append(c)
                    append_proj(s + 2)
                for tq in range(NT):
                    keep = 0 if (s == NB - 1 and tq == NT - 1) else 2
                    while len(pending) > keep:
                        ps_, ptq = pending.pop(0)
                        fillers.append(norm(ps_, ptq))
                        for m in range(4 * ptq, 4 * ptq + 4):
                            fillers.append(outproj(ps_, m))
                    gen = attn_gen(s, tq)
                    for step in range(J + 2):
                        if step == 0:
                            req = [mk_pq.get((s, tq), 0),
                                   mk_pk.get((s, 0), 0)]
                        elif step <= J:
                            j = step - 1  # emits pv(j) then qk(j+2)
                            req = [mk_pv.get((s, j), 0)]
                            if j + 2 < J:
                                req.append(mk_pk.get((s, (j + 2) // 4), 0))
                        else:
                            req = []
                        drain_to(max(req, default=0))
                        next(gen)
                        done_attn += 1
                        acc += 1.3 * (len(fillers) - fi) / max(
                            1, total_attn - done_attn)
                        n = int(acc)
                        acc -= n
                        for _ in range(n):
                            if fi < len(fillers):
                                fillers[fi]()
                                fi += 1
                    pending.append((s, tq))
            while pending:
                ps_, ptq = pending.pop(0)
                fillers.append(norm(ps_, ptq))
                for m in range(4 * ptq, 4 * ptq + 4):
                    fillers.append(outproj(ps_, m))
            drain_to(len(fillers))

    _split_multi_waits(nc)
    return nc


_CACHE = {}


def _get_nc(NB, J_list, dt_x, dt_in):
    key = (NB, tuple(J_list), str(dt_x), str(dt_in))
    if key not in _CACHE:
        _CACHE[key] = build_nc(NB, J_list, dt_x, dt_in)
    return _CACHE[key]


def _xt(x, dt_np, nw):
    """[T, D] -> [nw, P, KC, 512] transposed window-major layout."""
    xt = x.T.reshape(KC, P, NT, 512).transpose(2, 1, 0, 3)[:nw]
    return np.ascontiguousarray(xt).astype(dt_np)


def kernel(**inputs):
    query = np.asarray(inputs["query"], dtype=np.float32)
    key = np.asarray(inputs["key"], dtype=np.float32)
    value = np.asarray(inputs["value"], dtype=np.float32)
    vl = np.asarray(inputs["valid_length"]).astype(np.int64)
    W_q = np.asarray(inputs["W_q"], dtype=np.float32)
    b_q = np.asarray(inputs["b_q"], dtype=np.float32)
    W_k = np.asarray(inputs["W_k"], dtype=np.float32)
    b_k = np.asarray(inputs["b_k"], dtype=np.float32)
    W_v = np.asarray(inputs["W_v"], dtype=np.float32)
    b_v = np.asarray(inputs["b_v"], dtype=np.float32)
    W_o = np.asarray(inputs["W_o"], dtype=np.float32)
    b_o = np.asarray(inputs["b_o"], dtype=np.float32)

    B = query.shape[0]
    NB = B
    CPB = (H // NCORES) * DK       # 2 heads per core -> 128 cols
    dt_x = BF16
    dt_np = BF16_NP
    dt_in = FP8E3
    din_np = FP8E3_NP

    # slot s handles batch order[s]; J (Tk tiles) baked per slot, asc order
    order = np.argsort(vl, kind="stable")
    # vl==0 -> uniform attention over all T keys (q zeroed); sort those last
    order = np.concatenate([order[vl[order] != 0], order[vl[order] == 0]])
    J_list = []
    for s in range(NB):
        v = int(vl[order[s]])
        J_list.append(TC if v == 0 else max(1, -(-v // P)))
    WK_list = [min(NT, -(-j * P // 512)) for j in J_list]

    nc = _get_nc(NB, J_list, dt_x, dt_in)

    # host-side shard prep (shared across cores)
    xq_np, xk_np, xv_np = [], [], []
    mk_np = np.zeros((P, NB), np.float32)
    for s in range(NB):
        b = int(order[s])
        v = int(vl[b])
        q_b = query[b] if v != 0 else np.zeros_like(query[b])
        xq_np.append(_xt(q_b, din_np, NT))
        xk_np.append(_xt(key[b], din_np, WK_list[s]))
        xv_np.append(_xt(value[b], din_np, WK_list[s]))
        if v != 0:
            # mask bias for the last Tk tile (rows j*128+p >= v)
            rows = (J_list[s] - 1) * P + np.arange(P)
            mk_np[:, s] = np.where(rows < v, 0.0, MASK_NEG)

    sel_np = np.zeros((2, P), np.float32)
    sel_np[0, 0:DK] = 1.0
    sel_np[1, DK:P] = 1.0
    in_maps = []
    for c in range(NCORES):
        c0 = c * CPB
        cols = slice(c0, c0 + CPB)
        im = {
            "wq": np.ascontiguousarray(
                W_q.reshape(KC, P, H * DK).transpose(1, 0, 2)[:, :, cols]
            ).astype(dt_np),
            "wk": np.ascontiguousarray(
                W_k.reshape(KC, P, H * DK).transpose(1, 0, 2)[:, :, cols]
            ).astype(dt_np),
            "wv": np.ascontiguousarray(
                W_v.reshape(KC, P, H * DK).transpose(1, 0, 2)[:, :, cols]
            ).astype(dt_np),
            "wo": np.ascontiguousarray(W_o[cols]).astype(dt_np),
            "bq": np.ascontiguousarray(b_q[cols][:, None]).astype(np.float32),
            "bk": np.ascontiguousarray(b_k[cols][:, None]).astype(np.float32),
            "bv": np.ascontiguousarray(b_v[cols][None, :]).astype(dt_np),
            "mk": mk_np,
            "sel": sel_np,
        }
        for s in range(NB):
            im[f"xq{s}"] = xq_np[s]
            im[f"xk{s}"] = xk_np[s]
            im[f"xv{s}"] = xv_np[s]
        in_maps.append(im)

    res = run_bass_kernel_spmd(nc, in_maps, list(range(NCORES)))

    out = np.zeros((B, T, D), np.float32)
    for s in range(NB):
        b = int(order[s])
        acc = np.zeros((T, D), np.float32)
        for c in range(NCORES):
            acc += np.asarray(res.results[c][f"o{s}"]).astype(np.float32)
        out[b] = acc + b_o[None, :]
    return out
